# revision 22
# baseline (speedup 1.0000x reference)
"""AttentionPooling kernel for 8 Trainium2 NeuronCores.

Computation (per graph g): out[g] = sum_i softmax(logits)_i * x_i over nodes i in g,
where logits = tanh(x @ W1 + b1) @ W2 + b2.

Strategy (v2):
- logits are bounded (|logit| <= sum|W2| + |b2| < 17), so exp() is safe without the
  max-subtraction: w_i = e_i / sum(e) with e_i = exp(logit_i). Single pass over x.
- Shard 8192 graphs across 8 cores (1024 each). Per core, 8 "graph blocks" of 128
  graphs; a block's 128 graphs map to the 128 PSUM partitions of an accumulator.
- batch is known on host: node ranges per block are computed on host and the x rows
  are gathered per (core, block) into fixed-size slabs of T_blk*128 rows, so all 8
  cores run one identical program (SPMD).
- x is shipped twice in DMA-friendly layouts: once transposed+packed fp8e4m3
  [128, L, 2] for the MLP matmul (DoubleRow perf mode contracts K=256 in one
  matmul at 0.5 cyc/row), once as [128, T_tot, 257] bf16 node-major (with ones
  column) for the pooling matmul. Both give >=2KB contiguous DMA lines/partition.
- W1 is scaled by 16 into fp8e4m3 normal range; the 1/16 is folded into the tanh
  activation's scale operand.
- Per group of 8 subtiles (1024 nodes): 4 DoubleRow matmuls -> hT in PSUM
  [128, 1024] x2 halves; 2 wide tanh ACTs -> th bf16; 16 tiny lg matmuls
  (th chunk stationary, W2 half moving) -> logit column [128 nodes, 1] each;
  exp ACT [128, 8] -> e; per subtile: onehot_e = (iota==bc)*e (DVE, all-bf16
  2x mode) and numer[g, 0:257] += onehot_e.T @ [x | 1] (PE, PSUM accum; col 256
  is the softmax denominator). Block epilogue divides and DMAs out.
- The group loop is software-pipelined 5 stages deep (DMA / W1 / tanh / lg /
  exp+oh+numer) so every engine's in-order queue always has ready work.
"""

import math
import os
from contextlib import ExitStack

import numpy as np
import ml_dtypes

try:
    import concourse.bass as bass
except ImportError:  # fallback if PYTHONPATH lacks the repo
    import sys

    sys.path.insert(0, "/opt/trn_rl_repo")
    import concourse.bass as bass

import concourse.tile as tile
from concourse import bass_utils, mybir

BF16 = ml_dtypes.bfloat16
F8 = ml_dtypes.float8_e4m3
F32 = np.float32

LAST_EXEC_NS = None
LAST_TRACE_PATH = None

N_CORES = 8
N_NODES = 1_000_000
H = 256  # hidden
G = 8192  # num graphs
GPC = G // N_CORES  # graphs per core = 1024
GPB = 128  # graphs per block (= PSUM partitions)
BPC = GPC // GPB  # blocks per core = 8
P = 128  # partitions / nodes per subtile
GRP = 8  # subtiles per group (1024 nodes)
W1_SCALE = 16.0


def _split_sync_waits(nc, maxw: int = 1) -> int:
    """The walrus build in this container rejects instructions carrying more
    than one sync-wait. Hoist extra waits onto NoOps inserted just before the
    instruction (same engine, same order => identical semantics)."""
    cnt = 0
    for f in nc.m.functions:
        for bb in f.blocks:
            insts = bb.instructions
            out = []
            changed = False
            for ins in insts:
                si = ins.sync_info
                if si is not None and len(si.on_wait) > maxw:
                    waits = list(si.on_wait)
                    keep, extra = waits[-maxw:], waits[:-maxw]
                    for w in extra:
                        cnt += 1
                        nop = mybir.InstNoOp(
                            name=f"wsplit-{cnt}",
                            engine=ins.engine,
                            sync_info=mybir.SyncInfo(on_wait=[w], on_update=[]),
                            bass_nofuse=True,
                        )
                        nc.register_instruction(nop, overwrite=True)
                        out.append(nop)
                    ins.sync_info = mybir.SyncInfo(
                        on_wait=keep, on_update=si.on_update
                    )
                    changed = True
                out.append(ins)
            if changed:
                bb.instructions = out
    return cnt


def _build_program(T_blk: int):
    nc = bass.Bass("TRN2", target_bir_lowering=False)
    T_tot = BPC * T_blk  # subtiles per core; divisible by GRP since BPC=8
    L = T_tot * P  # node slots per core
    NG = T_tot // GRP  # groups per core
    assert T_tot % GRP == 0

    f32 = mybir.dt.float32
    bf16 = mybir.dt.bfloat16
    f8e4 = mybir.dt.float8e4

    xt8_d = nc.declare_dram_parameter("xt8", [P, L, 2], f8e4, isOutput=False)
    xn_d = nc.declare_dram_parameter("xn", [P, T_tot, H + 1], bf16, isOutput=False)
    bc_d = nc.declare_dram_parameter("bc", [P, T_tot], f32, isOutput=False)
    w18a_d = nc.declare_dram_parameter("w18a", [P, 2, P], f8e4, isOutput=False)
    w18b_d = nc.declare_dram_parameter("w18b", [P, 2, P], f8e4, isOutput=False)
    w2a_d = nc.declare_dram_parameter("w2a", [P, 1], bf16, isOutput=False)
    w2b_d = nc.declare_dram_parameter("w2b", [P, 1], bf16, isOutput=False)
    b1a_d = nc.declare_dram_parameter("b1a", [P, 1], f32, isOutput=False)
    b1b_d = nc.declare_dram_parameter("b1b", [P, 1], f32, isOutput=False)
    b2c_d = nc.declare_dram_parameter("b2c", [P, 1], f32, isOutput=False)
    iota_d = nc.declare_dram_parameter("iota", [P, P], bf16, isOutput=False)
    out_d = nc.declare_dram_parameter("out", [GPC, H], f32, isOutput=True)

    Tanh = mybir.ActivationFunctionType.Tanh
    Exp = mybir.ActivationFunctionType.Exp
    EQ = mybir.AluOpType.is_equal
    MUL = mybir.AluOpType.mult
    ADD = mybir.AluOpType.add
    DR = mybir.MatmulPerfMode.DoubleRow

    NGRP = GRP * P  # nodes per group = 1024

    with tile.TileContext(nc) as tc:
        with ExitStack() as ctx:
            consts = ctx.enter_context(tc.tile_pool(name="consts", bufs=1))
            xtpool = ctx.enter_context(tc.tile_pool(name="xt", bufs=4))
            xnpool = ctx.enter_context(tc.tile_pool(name="xn", bufs=8))
            thpool = ctx.enter_context(tc.tile_pool(name="th", bufs=4))
            epool = ctx.enter_context(tc.tile_pool(name="e", bufs=4))
            ohpool = ctx.enter_context(tc.tile_pool(name="oh", bufs=20))
            outpool = ctx.enter_context(tc.tile_pool(name="outp", bufs=2))
            # PSUM budget (8 banks of 2KB): ht 1 buf x 2 tags x [128,1024]f32
            # (2 banks each) = 4, numer 2 x [128,257]f32 = 2, lg 2 x [128,8] = 2.
            ps_ht = ctx.enter_context(
                tc.tile_pool(name="ps_ht", bufs=1, space=bass.MemorySpace.PSUM)
            )
            ps_lg = ctx.enter_context(
                tc.tile_pool(name="ps_lg", bufs=2, space=bass.MemorySpace.PSUM)
            )
            ps_nm = ctx.enter_context(
                tc.tile_pool(name="ps_nm", bufs=2, space=bass.MemorySpace.PSUM)
            )

            # ---- constants (loaded once) ----
            w18a_t = consts.tile([P, 2, P], f8e4)
            nc.sync.dma_start(w18a_t[:], w18a_d[:])
            w18b_t = consts.tile([P, 2, P], f8e4)
            nc.sync.dma_start(w18b_t[:], w18b_d[:])
            w2a_t = consts.tile([P, 1], bf16)
            nc.sync.dma_start(w2a_t[:], w2a_d[:])
            w2b_t = consts.tile([P, 1], bf16)
            nc.sync.dma_start(w2b_t[:], w2b_d[:])
            b1a_t = consts.tile([P, 1], f32)
            nc.sync.dma_start(b1a_t[:], b1a_d[:])
            b1b_t = consts.tile([P, 1], f32)
            nc.sync.dma_start(b1b_t[:], b1b_d[:])
            b2c_t = consts.tile([P, 1], f32)
            nc.sync.dma_start(b2c_t[:], b2c_d[:])
            iota_t = consts.tile([P, P], bf16)
            nc.sync.dma_start(iota_t[:], iota_d[:])
            bc_t = consts.tile([P, T_tot], f32)
            nc.sync.dma_start(bc_t[:], bc_d[:])

            xt_tiles = {}
            xn_tiles = {}
            ht_tiles = {}
            th_tiles = {}
            lg_tiles = {}
            oh_tiles = {}
            numer_ref = [None]

            def stage_dma(g):
                xt8 = xtpool.tile([P, NGRP, 2], f8e4, tag="xt8")
                nc.sync.dma_start(xt8[:], xt8_d[:, g * NGRP : (g + 1) * NGRP, :])
                xt_tiles[g] = xt8
                xnt = xnpool.tile([P, GRP, H + 1], bf16, tag="xnt")
                nc.sync.dma_start(xnt[:], xn_d[:, g * GRP : (g + 1) * GRP, :])
                xn_tiles[g] = xnt

            def stage_w1A(g):
                xt8 = xt_tiles[g]
                htA = ps_ht.tile([P, NGRP], f32, tag="htA")
                ht_tiles[("A", g)] = htA
                for q in range(2):
                    s, e = q * 512, (q + 1) * 512
                    rhs = xt8[:, s:e, :].rearrange("p n i -> p i n")
                    nc.tensor.matmul(
                        htA[:, s:e], w18a_t[:], rhs,
                        start=True, stop=True, perf_mode=DR,
                        skip_group_check=True,
                    )

            def stage_w1B(g):
                xt8 = xt_tiles.pop(g)
                htB = ps_ht.tile([P, NGRP], f32, tag="htB")
                ht_tiles[("B", g)] = htB
                for q in range(2):
                    s, e = q * 512, (q + 1) * 512
                    rhs = xt8[:, s:e, :].rearrange("p n i -> p i n")
                    nc.tensor.matmul(
                        htB[:, s:e], w18b_t[:], rhs,
                        start=True, stop=True, perf_mode=DR,
                        skip_group_check=True,
                    )

            def stage_tanhA(g):
                htA = ht_tiles.pop(("A", g))
                thA = thpool.tile([P, NGRP], bf16, tag="thA")
                nc.scalar.activation(
                    thA[:], htA[:], Tanh, bias=b1a_t[:], scale=1.0 / W1_SCALE
                )
                th_tiles[("A", g)] = thA

            def stage_tanhB(g):
                htB = ht_tiles.pop(("B", g))
                thB = thpool.tile([P, NGRP], bf16, tag="thB")
                nc.scalar.activation(
                    thB[:], htB[:], Tanh, bias=b1b_t[:], scale=1.0 / W1_SCALE
                )
                th_tiles[("B", g)] = thB

            def stage_lg(g):
                thA = th_tiles.pop(("A", g))
                thB = th_tiles.pop(("B", g))
                lg = ps_lg.tile([P, GRP], f32, tag="lg")
                lg_tiles[g] = lg
                for ii in range(GRP):
                    nc.tensor.matmul(
                        lg[:, ii : ii + 1],
                        thA[:, ii * P : (ii + 1) * P],
                        w2a_t[:],
                        start=True, stop=False, skip_group_check=True,
                    )
                    nc.tensor.matmul(
                        lg[:, ii : ii + 1],
                        thB[:, ii * P : (ii + 1) * P],
                        w2b_t[:],
                        start=False, stop=True, skip_group_check=True,
                    )

            def stage_exp_oh(g):
                lg = lg_tiles.pop(g)
                ecols = epool.tile([P, GRP], f32, tag="ecols")
                nc.scalar.activation(ecols[:], lg[:], Exp, bias=b2c_t[:])
                ohs = []
                for gi in range(GRP):
                    j = g * GRP + gi
                    oh = ohpool.tile([P, P], bf16, tag="oh")
                    # The DVE runs these at ~222ns cadence (2x mode doesn't
                    # engage) and paces the numer chain; GpSimd sits idle.
                    # Alternate between the two so their combined cadence
                    # keeps the PE's numer matmuls fed.
                    eng = nc.vector if gi % 2 == 0 else nc.gpsimd
                    eng.tensor_scalar(
                        oh[:], iota_t[:], bc_t[:, j : j + 1],
                        ecols[:, gi : gi + 1], EQ, MUL,
                    )
                    ohs.append(oh)
                return ohs

            def stage_numer(g, ohs):
                xnt = xn_tiles.pop(g)
                for gi in range(GRP):
                    j = g * GRP + gi
                    blk, t_in = divmod(j, T_blk)
                    if t_in == 0:
                        numer = ps_nm.tile([P, H + 1], f32, tag="numer")
                        numer_ref[0] = numer
                    numer = numer_ref[0]
                    nc.tensor.matmul(
                        numer[:],
                        ohs[gi][:],
                        xnt[:, gi, :],
                        start=(t_in == 0),
                        stop=(t_in == T_blk - 1),
                        skip_group_check=True,
                    )
                    if t_in == T_blk - 1:
                        # block epilogue: out[g] = numer[g, :256] / numer[g, 256]
                        dn = epool.tile([P, 1], f32, tag="dn")
                        nc.vector.tensor_scalar(
                            dn[:], numer[:, H : H + 1], 1e-30, None, ADD
                        )
                        rec = epool.tile([P, 1], f32, tag="rec")
                        nc.vector.reciprocal(rec[:], dn[:])
                        outt = outpool.tile([P, H], f32, tag="outt")
                        nc.vector.tensor_scalar(
                            outt[:], numer[:, 0:H], rec[:], None, MUL
                        )
                        nc.sync.dma_start(
                            out_d[blk * GPB : (blk + 1) * GPB, :], outt[:]
                        )

            # ---- software-pipelined group loop ----
            # Half-staggered: the B half of each group runs one step behind
            # the A half so that no step's W1 matmuls wait on a tanh issued
            # in the same step (single-buffered PSUM h tiles).
            for s in range(NG + 6):
                gN = s - 6  # numer matmuls (+ block epilogue)
                gE = s - 5  # exp + oh
                gL = s - 4  # lg matmuls
                gTB = s - 3  # tanh half B
                gTA = s - 2  # tanh half A
                gWB = s - 2  # W1 half B
                gWA = s - 1  # W1 half A
                gD = s  # DMA in

                if gE >= 0 and gE < NG:
                    oh_tiles[gE] = stage_exp_oh(gE)
                if gN >= 0:
                    stage_numer(gN, oh_tiles.pop(gN))
                if gL >= 0 and gL < NG:
                    stage_lg(gL)
                if gWA >= 0 and gWA < NG:
                    stage_w1A(gWA)
                if gWB >= 0 and gWB < NG:
                    stage_w1B(gWB)
                if gTA >= 0 and gTA < NG:
                    stage_tanhA(gTA)
                if gTB >= 0 and gTB < NG:
                    stage_tanhB(gTB)
                if gD < NG:
                    stage_dma(gD)

    return nc


def _install_ntff_hook_shim():
    """This image's antenv lacks axon_hooks, so bass_utils' trace=True path
    can't find the NTFF profile hook trn_boot would register. Provide the
    module and register the ctypes hook ourselves (trace runs only)."""
    import sys as _sys
    import types as _types

    if "antenv.axon_hooks" in _sys.modules:
        return
    import antenv

    mod = _types.ModuleType("antenv.axon_hooks")
    mod._hook = None

    def set_axon_ntff_profile_hook(h):
        mod._hook = h

    def get_axon_ntff_profile_hook():
        return mod._hook

    mod.set_axon_ntff_profile_hook = set_axon_ntff_profile_hook
    mod.get_axon_ntff_profile_hook = get_axon_ntff_profile_hook
    _sys.modules["antenv.axon_hooks"] = mod
    antenv.axon_hooks = mod

    from trn_agent_boot.trn_boot import _ntff_profile_via_ctypes

    hook = _ntff_profile_via_ctypes("/opt/axon/libaxon_pjrt.so")
    if hook is not None:
        set_axon_ntff_profile_hook(hook)


def _run_spmd_retry(nc, in_maps, core_ids, label, tries=6, delay=25.0, **kw):
    """The tunneled device intermittently reports NRT_EXEC_UNIT_UNRECOVERABLE
    right after a previous process's close; it self-recovers within ~1-2 min.
    Retry with backoff instead of dying."""
    import time as _time

    for attempt in range(tries):
        try:
            return bass_utils.run_bass_kernel_spmd(nc, in_maps, core_ids, **kw)
        except Exception as e:  # noqa: BLE001
            msg = str(e)
            transient = (
                "UNRECOVERABLE" in msg
                or "UNAVAILABLE" in msg
                or "NRT_TIMEOUT" in msg
                or "PassThrough failed" in msg
            )
            if not transient or attempt == tries - 1:
                raise
            print(
                f"[kernel] {label}: transient device error "
                f"(attempt {attempt+1}/{tries}), retrying in {delay:.0f}s",
                flush=True,
            )
            _time.sleep(delay)
    raise RuntimeError("unreachable")


def _run_warmup():
    """Run a tiny NEFF touching every engine/op first. The first NEFF executed
    in a fresh process has been observed to hang when it contains the full
    pipeline (ACT table staging race?); a small warmup run avoids it."""
    f32 = mybir.dt.float32
    Tanh = mybir.ActivationFunctionType.Tanh
    Exp = mybir.ActivationFunctionType.Exp
    EQ = mybir.AluOpType.is_equal
    MUL = mybir.AluOpType.mult
    nc = bass.Bass("TRN2", target_bir_lowering=False)
    x_d = nc.declare_dram_parameter("x", [P, P], f32, isOutput=False)
    y_d = nc.declare_dram_parameter("y", [P, P], f32, isOutput=True)
    with tile.TileContext(nc) as tc:
        with ExitStack() as ctx:
            pool = ctx.enter_context(tc.tile_pool(name="p", bufs=2))
            ps = ctx.enter_context(
                tc.tile_pool(name="ps", bufs=1, space=bass.MemorySpace.PSUM)
            )
            t = pool.tile([P, P], f32)
            nc.sync.dma_start(t[:], x_d[:])
            acc = ps.tile([P, P], f32)
            nc.tensor.matmul(acc[:], t[:], t[:], start=True, stop=True)
            t2 = pool.tile([P, P], f32)
            nc.scalar.activation(t2[:], acc[:], Tanh, bias=t[:, 0:1])
            t3 = pool.tile([P, P], f32)
            nc.scalar.activation(t3[:], t2[:], Exp, bias=t[:, 0:1])
            t4 = pool.tile([P, P], f32)
            nc.vector.tensor_scalar(t4[:], t3[:], t[:, 0:1], t[:, 1:2], EQ, MUL)
            t5 = pool.tile([P, 1], f32)
            nc.vector.reciprocal(t5[:], t3[:, 0:1])
            nc.vector.tensor_scalar(t4[:, 0:1], t5[:], t5[:], None, MUL)
            nc.sync.dma_start(y_d[:], t4[:])
    _split_sync_waits(nc)
    xw = np.zeros((P, P), np.float32)
    _run_spmd_retry(
        nc, [{"x": xw} for _ in range(N_CORES)], list(range(N_CORES)), "warmup"
    )


def prepare_inputs(x, batch, W1, b1, W2, b2):
    """Host-side segmentation + per-core gather. Returns (T_blk, in_maps)."""
    x = np.asarray(x, dtype=F32)
    batch = np.asarray(batch).astype(np.int64)
    W1 = np.asarray(W1, dtype=F32)
    b1 = np.asarray(b1, dtype=F32)
    W2 = np.asarray(W2, dtype=F32)
    b2 = np.asarray(b2, dtype=F32)
    assert x.shape == (N_NODES, H) and batch.shape == (N_NODES,)

    # ---- host-side segmentation ----
    block_starts = np.searchsorted(batch, np.arange(0, G + 1, GPB)).astype(np.int64)
    cnts = np.diff(block_starts)
    T_blk = max(1, int(math.ceil(cnts.max() / P)))
    T_tot = BPC * T_blk
    L = T_tot * P

    import time as _time

    _tg = _time.time()
    # full-array dtype conversions once (fast contiguous casts)
    x16 = x.astype(BF16)
    x8 = x.astype(F8)

    xt8_all = []
    xn_all = []
    bc_all = []
    for c in range(N_CORES):
        x16_pad = np.zeros((L, H), dtype=BF16)
        x8_pad = np.zeros((L, H), dtype=F8)
        bc_c = np.full((P, T_tot), -1.0, dtype=F32)
        for b in range(BPC):
            gblk = c * BPC + b
            s = int(block_starts[gblk])
            e = min(s + T_blk * P, N_NODES)
            n = e - s
            if n <= 0:
                continue
            r0 = b * T_blk * P
            x16_pad[r0 : r0 + n] = x16[s:e]
            x8_pad[r0 : r0 + n] = x8[s:e]
            vals = np.full(T_blk * P, -1.0, dtype=F32)
            vals[:n] = (batch[s:e] - gblk * GPB).astype(F32)
            bc_c[:, b * T_blk : (b + 1) * T_blk] = vals.reshape(T_blk, P).T
        # xn layout [128, T_tot, 257]: row (p, t) = [x[node t*128+p], 1.0]
        xn_c = np.ones((P, T_tot, H + 1), dtype=BF16)
        xn_c[:, :, 0:H] = x16_pad.reshape(T_tot, P, H).transpose(1, 0, 2)
        # xt8 layout [128, L, 2]: xt8[p, n, i] = x8[n, 128i + p]
        xt8_c = np.ascontiguousarray(
            x8_pad.reshape(L, 2, P).transpose(2, 0, 1)
        )
        xt8_all.append(xt8_c)
        xn_all.append(xn_c)
        bc_all.append(bc_c)
    print(f"[kernel] host gather: {_time.time()-_tg:.1f}s", flush=True)

    W1s = (W1 * W1_SCALE).astype(F8)  # [256, 256], scaled into e4m3 range
    w18 = W1s.reshape(2, P, H).transpose(1, 0, 2)  # [p, i, m_full]
    consts = {
        "w18a": np.ascontiguousarray(w18[:, :, 0:P]),
        "w18b": np.ascontiguousarray(w18[:, :, P:H]),
        "w2a": W2[0:P, :].astype(BF16),
        "w2b": W2[P:H, :].astype(BF16),
        "b1a": b1[0:P, None].astype(F32),
        "b1b": b1[P:H, None].astype(F32),
        "b2c": np.full((P, 1), b2[0] if b2.ndim else b2, dtype=F32),
        "iota": np.tile(np.arange(P, dtype=F32), (P, 1)).astype(BF16),
    }

    in_maps = [
        {"xt8": xt8_all[c], "xn": xn_all[c], "bc": bc_all[c], **consts}
        for c in range(N_CORES)
    ]
    return T_blk, in_maps


def kernel(x, batch, num_graphs, W1, b1, W2, b2):
    import time as _time

    ng = int(num_graphs)
    assert ng == G
    T_blk, in_maps = prepare_inputs(x, batch, W1, b1, W2, b2)

    t0 = _time.time()
    nc = _build_program(T_blk)
    _split_sync_waits(nc)
    print(f"[kernel] build+split: {_time.time()-t0:.1f}s (T_blk={T_blk})", flush=True)

    t0 = _time.time()
    _run_warmup()
    print(f"[kernel] warmup run: {_time.time()-t0:.1f}s", flush=True)

    t0 = _time.time()
    trace = os.environ.get("KERNEL_TRACE", "0") == "1"
    if trace:
        _install_ntff_hook_shim()
    res = _run_spmd_retry(
        nc, in_maps, list(range(N_CORES)), "main", trace=trace
    )
    print(f"[kernel] main run (compile+upload+exec): {_time.time()-t0:.1f}s", flush=True)
    if trace:
        global LAST_EXEC_NS, LAST_TRACE_PATH
        LAST_EXEC_NS = res.exec_time_ns
        if res.instructions_and_trace is not None:
            LAST_TRACE_PATH = res.instructions_and_trace[1]
        print(f"[kernel] exec_time_ns={res.exec_time_ns} trace={LAST_TRACE_PATH}",
              flush=True)

    out = np.concatenate([res.results[c]["out"] for c in range(N_CORES)], axis=0)
    return out.astype(F32)


# revision 43
# speedup vs baseline: 4.0060x; 4.0060x over previous
"""AttentionPooling kernel for 8 Trainium2 NeuronCores.

Computation (per graph g): out[g] = sum_i softmax(logits)_i * x_i over nodes i in g,
where logits = tanh(x @ W1 + b1) @ W2 + b2.

Strategy:
- logits are bounded (|logit| <= sum|W2| + |b2| < 17), so exp() is safe without the
  max-subtraction: w_i = e_i / sum(e) with e_i = exp(logit_i). Single pass over x.
- Shard 8192 graphs across 8 cores (1024 each). Per core, 8 "graph blocks" of 128
  graphs; a block's 128 graphs map to the 128 PSUM partitions of an accumulator.
- batch is known on host: node ranges per block are computed on host and the x rows
  are gathered per (core, block) into fixed-size slabs of T_blk*128 rows, so all 8
  cores run one identical program (SPMD).
- x is shipped twice, fp8 both times (68MB total vs 131MB for the bf16 baseline),
  in DMA-friendly layouts (>=2KB contiguous lines per partition, two groups per
  DMA instruction): transposed+packed fp8e4m3 [128, L, 2] for the MLP matmul
  (DoubleRow perf mode contracts K=256 in one matmul), and fp8e3m4
  [128, T_tot, 257] node-major (with ones column) for the pooling matmul
  (PE takes mixed bf16 lhsT x fp8 rhs). Measured end-to-end rel err 1.69e-2
  (gate 2e-2), bit-matching the numpy simulation of the same quantization chain.
- W1 is scaled by 16 into fp8e4m3 normal range; the 1/16 is folded into the tanh
  activation's scale operand.
- Per group of 8 subtiles (1024 nodes): 4 DoubleRow matmuls -> hT in PSUM
  [128, 1024] x2 halves; 2 wide tanh ACTs -> th bf16; 16 tiny lg matmuls
  (th chunk stationary, W2 half moving) -> logit column [128 nodes, 1] each;
  exp ACT [128, 8] -> e; per subtile: onehot_e = (iota==bc)*e (DVE) and
  numer[g, 0:257] += onehot_e.T @ [x | 1] (PE, PSUM accum; col 256 is the
  softmax denominator). Block epilogue divides and DMAs out.
- The group loop is software-pipelined 8 steps deep (DMA / W1-A / W1-B /
  tanh-A / tanh-B / lg / exp / oh / numer) with the two h-halves staggered a
  step apart, so every engine's in-order queue has ready work each step and
  single-buffered PSUM h tiles never stall the PE behind the ACT.
"""

import math
import os
from contextlib import ExitStack

import numpy as np
import ml_dtypes

try:
    import concourse.bass as bass
except ImportError:  # fallback if PYTHONPATH lacks the repo
    import sys

    sys.path.insert(0, "/opt/trn_rl_repo")
    import concourse.bass as bass

import concourse.tile as tile
from concourse import bass_utils, mybir

BF16 = ml_dtypes.bfloat16
F8 = ml_dtypes.float8_e4m3
F8E3 = ml_dtypes.float8_e3m4
F32 = np.float32

LAST_EXEC_NS = None
LAST_TRACE_PATH = None

N_CORES = 8
N_NODES = 1_000_000
H = 256  # hidden
G = 8192  # num graphs
GPC = G // N_CORES  # graphs per core = 1024
GPB = 128  # graphs per block (= PSUM partitions)
BPC = GPC // GPB  # blocks per core = 8
P = 128  # partitions / nodes per subtile
GRP = 8  # subtiles per group (1024 nodes)
W1_SCALE = 16.0


def _split_sync_waits(nc, maxw: int = 1) -> int:
    """The walrus build in this container rejects instructions carrying more
    than one sync-wait. Hoist extra waits onto NoOps inserted just before the
    instruction (same engine, same order => identical semantics)."""
    cnt = 0
    for f in nc.m.functions:
        for bb in f.blocks:
            insts = bb.instructions
            out = []
            changed = False
            for ins in insts:
                si = ins.sync_info
                if si is not None and len(si.on_wait) > maxw:
                    waits = list(si.on_wait)
                    keep, extra = waits[-maxw:], waits[:-maxw]
                    for w in extra:
                        cnt += 1
                        nop = mybir.InstNoOp(
                            name=f"wsplit-{cnt}",
                            engine=ins.engine,
                            sync_info=mybir.SyncInfo(on_wait=[w], on_update=[]),
                            bass_nofuse=True,
                        )
                        nc.register_instruction(nop, overwrite=True)
                        out.append(nop)
                    ins.sync_info = mybir.SyncInfo(
                        on_wait=keep, on_update=si.on_update
                    )
                    changed = True
                out.append(ins)
            if changed:
                bb.instructions = out
    return cnt


def _build_program(T_blk: int):
    nc = bass.Bass("TRN2", target_bir_lowering=False)
    T_tot = BPC * T_blk  # subtiles per core; divisible by GRP since BPC=8
    L = T_tot * P  # node slots per core
    NG = T_tot // GRP  # groups per core
    assert T_tot % GRP == 0

    f32 = mybir.dt.float32
    bf16 = mybir.dt.bfloat16
    f8e4 = mybir.dt.float8e4
    f8e3 = mybir.dt.float8e3

    xt8_d = nc.declare_dram_parameter("xt8", [P, L, 2], f8e4, isOutput=False)
    xn_d = nc.declare_dram_parameter("xn", [P, T_tot, H + 1], f8e3, isOutput=False)
    bc_d = nc.declare_dram_parameter("bc", [P, T_tot], f32, isOutput=False)
    w18a_d = nc.declare_dram_parameter("w18a", [P, 2, P], f8e4, isOutput=False)
    w18b_d = nc.declare_dram_parameter("w18b", [P, 2, P], f8e4, isOutput=False)
    w2a_d = nc.declare_dram_parameter("w2a", [P, 1], bf16, isOutput=False)
    w2b_d = nc.declare_dram_parameter("w2b", [P, 1], bf16, isOutput=False)
    b1a_d = nc.declare_dram_parameter("b1a", [P, 1], f32, isOutput=False)
    b1b_d = nc.declare_dram_parameter("b1b", [P, 1], f32, isOutput=False)
    b2c_d = nc.declare_dram_parameter("b2c", [P, 1], f32, isOutput=False)
    iota_d = nc.declare_dram_parameter("iota", [P, P], bf16, isOutput=False)
    out_d = nc.declare_dram_parameter("out", [GPC, H], f32, isOutput=True)

    Tanh = mybir.ActivationFunctionType.Tanh
    Exp = mybir.ActivationFunctionType.Exp
    EQ = mybir.AluOpType.is_equal
    MUL = mybir.AluOpType.mult
    ADD = mybir.AluOpType.add
    DR = mybir.MatmulPerfMode.DoubleRow

    NGRP = GRP * P  # nodes per group = 1024

    with tile.TileContext(nc) as tc:
        with ExitStack() as ctx:
            consts = ctx.enter_context(tc.tile_pool(name="consts", bufs=1))
            xtpool = ctx.enter_context(tc.tile_pool(name="xt", bufs=4))
            xnpool = ctx.enter_context(tc.tile_pool(name="xn", bufs=9))
            thpool = ctx.enter_context(tc.tile_pool(name="th", bufs=6))
            epool = ctx.enter_context(tc.tile_pool(name="e", bufs=4))
            ohpool = ctx.enter_context(tc.tile_pool(name="oh", bufs=24))
            outpool = ctx.enter_context(tc.tile_pool(name="outp", bufs=2))
            # PSUM budget (8 banks of 2KB): ht 1 buf x 2 tags x [128,1024]f32
            # (2 banks each) = 4, numer 2 x [128,257]f32 = 2, lg 2 x [128,8] = 2.
            ps_ht = ctx.enter_context(
                tc.tile_pool(name="ps_ht", bufs=1, space=bass.MemorySpace.PSUM)
            )
            ps_lg = ctx.enter_context(
                tc.tile_pool(name="ps_lg", bufs=2, space=bass.MemorySpace.PSUM)
            )
            ps_nm = ctx.enter_context(
                tc.tile_pool(name="ps_nm", bufs=2, space=bass.MemorySpace.PSUM)
            )

            # ---- constants (loaded once) ----
            w18a_t = consts.tile([P, 2, P], f8e4)
            nc.sync.dma_start(w18a_t[:], w18a_d[:])
            w18b_t = consts.tile([P, 2, P], f8e4)
            nc.sync.dma_start(w18b_t[:], w18b_d[:])
            w2a_t = consts.tile([P, 1], bf16)
            nc.sync.dma_start(w2a_t[:], w2a_d[:])
            w2b_t = consts.tile([P, 1], bf16)
            nc.sync.dma_start(w2b_t[:], w2b_d[:])
            b1a_t = consts.tile([P, 1], f32)
            nc.sync.dma_start(b1a_t[:], b1a_d[:])
            b1b_t = consts.tile([P, 1], f32)
            nc.sync.dma_start(b1b_t[:], b1b_d[:])
            b2c_t = consts.tile([P, 1], f32)
            nc.sync.dma_start(b2c_t[:], b2c_d[:])
            iota_t = consts.tile([P, P], bf16)
            nc.sync.dma_start(iota_t[:], iota_d[:])
            bc_t = consts.tile([P, T_tot], f32)
            nc.sync.dma_start(bc_t[:], bc_d[:])

            xt_tiles = {}
            xn_tiles = {}
            ht_tiles = {}
            th_tiles = {}
            lg_tiles = {}
            lg_work = {}
            lg_pair = {}
            e_tiles = {}
            oh_tiles = {}
            numer_ref = [None]

            def stage_dma(g):
                # fetch two groups per DMA instruction (4KB contiguous
                # per-partition lines) to halve packet count
                w = 2 if g + 1 < NG else 1
                xt8 = xtpool.tile([P, 2 * NGRP, 2], f8e4, tag="xt8")
                nc.sync.dma_start(
                    xt8[:, 0 : w * NGRP, :],
                    xt8_d[:, g * NGRP : (g + w) * NGRP, :],
                )
                xnt = xnpool.tile([P, 2 * GRP, H + 1], f8e3, tag="xnt")
                nc.sync.dma_start(
                    xnt[:, 0 : w * GRP, :],
                    xn_d[:, g * GRP : (g + w) * GRP, :],
                )
                for k in range(w):
                    xt_tiles[g + k] = xt8[:, k * NGRP : (k + 1) * NGRP, :]
                    xn_tiles[g + k] = xnt[:, k * GRP : (k + 1) * GRP, :]

            def stage_w1A(g):
                xt8 = xt_tiles[g]
                htA = ps_ht.tile([P, NGRP], f32, tag="htA")
                ht_tiles[("A", g)] = htA
                for q in range(2):
                    s, e = q * 512, (q + 1) * 512
                    rhs = xt8[:, s:e, :].rearrange("p n i -> p i n")
                    nc.tensor.matmul(
                        htA[:, s:e], w18a_t[:], rhs,
                        start=True, stop=True, perf_mode=DR,
                        skip_group_check=True,
                    )

            def stage_w1B(g):
                xt8 = xt_tiles.pop(g)
                htB = ps_ht.tile([P, NGRP], f32, tag="htB")
                ht_tiles[("B", g)] = htB
                for q in range(2):
                    s, e = q * 512, (q + 1) * 512
                    rhs = xt8[:, s:e, :].rearrange("p n i -> p i n")
                    nc.tensor.matmul(
                        htB[:, s:e], w18b_t[:], rhs,
                        start=True, stop=True, perf_mode=DR,
                        skip_group_check=True,
                    )

            def stage_tanhA(g):
                htA = ht_tiles.pop(("A", g))
                thA = thpool.tile([P, NGRP], bf16, tag="thA")
                nc.scalar.activation(
                    thA[:], htA[:], Tanh, bias=b1a_t[:], scale=1.0 / W1_SCALE
                )
                th_tiles[("A", g)] = thA

            def stage_tanhB(g):
                htB = ht_tiles.pop(("B", g))
                thB = thpool.tile([P, NGRP], bf16, tag="thB")
                nc.scalar.activation(
                    thB[:], htB[:], Tanh, bias=b1b_t[:], scale=1.0 / W1_SCALE
                )
                th_tiles[("B", g)] = thB

            def emit_lg(g, ii):
                thA, thB, lg = lg_work[g]
                nc.tensor.matmul(
                    lg[:, ii : ii + 1],
                    thA[:, ii * P : (ii + 1) * P],
                    w2a_t[:],
                    start=True, stop=False, skip_group_check=True,
                )
                nc.tensor.matmul(
                    lg[:, ii : ii + 1],
                    thB[:, ii * P : (ii + 1) * P],
                    w2b_t[:],
                    start=False, stop=True, skip_group_check=True,
                )

            def stage_lg_prep(g):
                thA = th_tiles.pop(("A", g))
                thB = th_tiles.pop(("B", g))
                lg = ps_lg.tile([P, GRP], f32, tag="lg")
                lg_tiles[g] = lg
                lg_work[g] = (thA, thB, lg)

            def stage_exp(g):
                lg = lg_tiles.pop(g)
                ecols = epool.tile([P, GRP], f32, tag="ecols")
                nc.scalar.activation(ecols[:], lg[:], Exp, bias=b2c_t[:])
                e_tiles[g] = ecols

            def stage_oh(g):
                ecols = e_tiles.pop(g)
                ohs = []
                for gi in range(GRP):
                    j = g * GRP + gi
                    oh = ohpool.tile([P, P], bf16, tag="oh")
                    nc.vector.tensor_scalar(
                        oh[:], iota_t[:], bc_t[:, j : j + 1],
                        ecols[:, gi : gi + 1], EQ, MUL,
                    )
                    ohs.append(oh)
                return ohs

            def stage_numer(g, ohs, g_lg):
                """Numer matmuls for group g, with group g_lg's tiny lg
                matmuls interleaved between them: an accumulating matmul's
                next LDWEIGHTS otherwise waits for the array to drain
                (~2x the stream time); the short lg matmuls fill those
                drain gaps with useful PE work."""
                xnt = xn_tiles.pop(g)
                for gi in range(GRP):
                    j = g * GRP + gi
                    blk, t_in = divmod(j, T_blk)
                    if t_in == 0:
                        numer = ps_nm.tile([P, H + 1], f32, tag="numer")
                        numer_ref[0] = numer
                    numer = numer_ref[0]
                    nc.tensor.matmul(
                        numer[:],
                        ohs[gi][:],
                        xnt[:, gi, :],
                        start=(t_in == 0),
                        stop=(t_in == T_blk - 1),
                        skip_group_check=True,
                    )
                    if g_lg is not None:
                        emit_lg(g_lg, gi)
                    if t_in == T_blk - 1:
                        # block epilogue: out[g] = numer[g, :256] / numer[g, 256]
                        dn = epool.tile([P, 1], f32, tag="dn")
                        nc.vector.tensor_scalar(
                            dn[:], numer[:, H : H + 1], 1e-30, None, ADD
                        )
                        rec = epool.tile([P, 1], f32, tag="rec")
                        nc.vector.reciprocal(rec[:], dn[:])
                        outt = outpool.tile([P, H], f32, tag="outt")
                        nc.vector.tensor_scalar(
                            outt[:], numer[:, 0:H], rec[:], None, MUL
                        )
                        nc.sync.dma_start(
                            out_d[blk * GPB : (blk + 1) * GPB, :], outt[:]
                        )

            # ---- software-pipelined group loop ----
            # Half-staggered: the B half of each group runs one step behind
            # the A half so that no step's W1 matmuls wait on a tanh issued
            # in the same step (single-buffered PSUM h tiles). The oh stage
            # has its own step so the DVE starts each step with zero
            # unsatisfied dependencies (its ecols were produced last step).
            for s in range(NG + 8):
                gN = s - 8  # numer matmuls (+ block epilogue)
                gO = s - 7  # onehot*e on DVE
                gE = s - 5  # exp
                gL = s - 4  # lg matmuls
                gTB = s - 3  # tanh half B
                gTA = s - 2  # tanh half A
                gWB = s - 2  # W1 half B
                gWA = s - 1  # W1 half A
                gD = s  # DMA in

                if gO >= 0 and gO < NG:
                    oh_tiles[gO] = stage_oh(gO)
                if gE >= 0 and gE < NG:
                    stage_exp(gE)
                has_lg = gL >= 0 and gL < NG
                if has_lg:
                    stage_lg_prep(gL)
                if gN >= 0:
                    stage_numer(gN, oh_tiles.pop(gN), gL if has_lg else None)
                elif has_lg:
                    for ii in range(GRP):
                        emit_lg(gL, ii)
                if has_lg:
                    lg_work.pop(gL)
                if gWA >= 0 and gWA < NG:
                    stage_w1A(gWA)
                if gWB >= 0 and gWB < NG:
                    stage_w1B(gWB)
                if gTA >= 0 and gTA < NG:
                    stage_tanhA(gTA)
                if gTB >= 0 and gTB < NG:
                    stage_tanhB(gTB)
                if gD < NG and gD % 2 == 0:
                    stage_dma(gD)

    return nc


def _install_ntff_hook_shim():
    """This image's antenv lacks axon_hooks, so bass_utils' trace=True path
    can't find the NTFF profile hook trn_boot would register. Provide the
    module and register the ctypes hook ourselves (trace runs only)."""
    import sys as _sys
    import types as _types

    if "antenv.axon_hooks" in _sys.modules:
        return
    import antenv

    mod = _types.ModuleType("antenv.axon_hooks")
    mod._hook = None

    def set_axon_ntff_profile_hook(h):
        mod._hook = h

    def get_axon_ntff_profile_hook():
        return mod._hook

    mod.set_axon_ntff_profile_hook = set_axon_ntff_profile_hook
    mod.get_axon_ntff_profile_hook = get_axon_ntff_profile_hook
    _sys.modules["antenv.axon_hooks"] = mod
    antenv.axon_hooks = mod

    from trn_agent_boot.trn_boot import _ntff_profile_via_ctypes

    hook = _ntff_profile_via_ctypes("/opt/axon/libaxon_pjrt.so")
    if hook is not None:
        set_axon_ntff_profile_hook(hook)


def _run_spmd_retry(nc, in_maps, core_ids, label, tries=6, delay=25.0, **kw):
    """The tunneled device intermittently reports NRT_EXEC_UNIT_UNRECOVERABLE
    right after a previous process's close; it self-recovers within ~1-2 min.
    Retry with backoff instead of dying."""
    import time as _time

    for attempt in range(tries):
        try:
            return bass_utils.run_bass_kernel_spmd(nc, in_maps, core_ids, **kw)
        except Exception as e:  # noqa: BLE001
            msg = str(e)
            transient = (
                "UNRECOVERABLE" in msg
                or "UNAVAILABLE" in msg
                or "NRT_TIMEOUT" in msg
                or "PassThrough failed" in msg
            )
            if not transient or attempt == tries - 1:
                raise
            print(
                f"[kernel] {label}: transient device error "
                f"(attempt {attempt+1}/{tries}), retrying in {delay:.0f}s",
                flush=True,
            )
            _time.sleep(delay)
    raise RuntimeError("unreachable")


def _run_warmup():
    """Run a tiny NEFF touching every engine/op first. The first NEFF executed
    in a fresh process has been observed to hang when it contains the full
    pipeline (ACT table staging race?); a small warmup run avoids it."""
    f32 = mybir.dt.float32
    Tanh = mybir.ActivationFunctionType.Tanh
    Exp = mybir.ActivationFunctionType.Exp
    EQ = mybir.AluOpType.is_equal
    MUL = mybir.AluOpType.mult
    nc = bass.Bass("TRN2", target_bir_lowering=False)
    x_d = nc.declare_dram_parameter("x", [P, P], f32, isOutput=False)
    y_d = nc.declare_dram_parameter("y", [P, P], f32, isOutput=True)
    with tile.TileContext(nc) as tc:
        with ExitStack() as ctx:
            pool = ctx.enter_context(tc.tile_pool(name="p", bufs=2))
            ps = ctx.enter_context(
                tc.tile_pool(name="ps", bufs=1, space=bass.MemorySpace.PSUM)
            )
            t = pool.tile([P, P], f32)
            nc.sync.dma_start(t[:], x_d[:])
            acc = ps.tile([P, P], f32)
            nc.tensor.matmul(acc[:], t[:], t[:], start=True, stop=True)
            t2 = pool.tile([P, P], f32)
            nc.scalar.activation(t2[:], acc[:], Tanh, bias=t[:, 0:1])
            t3 = pool.tile([P, P], f32)
            nc.scalar.activation(t3[:], t2[:], Exp, bias=t[:, 0:1])
            t4 = pool.tile([P, P], f32)
            nc.vector.tensor_scalar(t4[:], t3[:], t[:, 0:1], t[:, 1:2], EQ, MUL)
            t5 = pool.tile([P, 1], f32)
            nc.vector.reciprocal(t5[:], t3[:, 0:1])
            nc.vector.tensor_scalar(t4[:, 0:1], t5[:], t5[:], None, MUL)
            nc.sync.dma_start(y_d[:], t4[:])
    _split_sync_waits(nc)
    xw = np.zeros((P, P), np.float32)
    _run_spmd_retry(
        nc, [{"x": xw} for _ in range(N_CORES)], list(range(N_CORES)), "warmup"
    )


def prepare_inputs(x, batch, W1, b1, W2, b2):
    """Host-side segmentation + per-core gather. Returns (T_blk, in_maps)."""
    x = np.asarray(x, dtype=F32)
    batch = np.asarray(batch).astype(np.int64)
    W1 = np.asarray(W1, dtype=F32)
    b1 = np.asarray(b1, dtype=F32)
    W2 = np.asarray(W2, dtype=F32)
    b2 = np.asarray(b2, dtype=F32)
    assert x.shape == (N_NODES, H) and batch.shape == (N_NODES,)

    # ---- host-side segmentation ----
    block_starts = np.searchsorted(batch, np.arange(0, G + 1, GPB)).astype(np.int64)
    cnts = np.diff(block_starts)
    T_blk = max(1, int(math.ceil(cnts.max() / P)))
    T_tot = BPC * T_blk
    L = T_tot * P

    import time as _time

    _tg = _time.time()
    # full-array dtype conversions once (fast contiguous casts)
    x83 = x.astype(F8E3)
    x8 = x.astype(F8)

    xt8_all = []
    xn_all = []
    bc_all = []
    for c in range(N_CORES):
        x83_pad = np.zeros((L, H), dtype=F8E3)
        x8_pad = np.zeros((L, H), dtype=F8)
        bc_c = np.full((P, T_tot), -1.0, dtype=F32)
        for b in range(BPC):
            gblk = c * BPC + b
            s = int(block_starts[gblk])
            e = min(s + T_blk * P, N_NODES)
            n = e - s
            if n <= 0:
                continue
            r0 = b * T_blk * P
            x83_pad[r0 : r0 + n] = x83[s:e]
            x8_pad[r0 : r0 + n] = x8[s:e]
            vals = np.full(T_blk * P, -1.0, dtype=F32)
            vals[:n] = (batch[s:e] - gblk * GPB).astype(F32)
            bc_c[:, b * T_blk : (b + 1) * T_blk] = vals.reshape(T_blk, P).T
        # xn layout [128, T_tot, 257]: row (p, t) = [x[node t*128+p], 1.0]
        xn_c = np.ones((P, T_tot, H + 1), dtype=F8E3)
        xn_c[:, :, 0:H] = x83_pad.reshape(T_tot, P, H).transpose(1, 0, 2)
        # xt8 layout [128, L, 2]: xt8[p, n, i] = x8[n, 128i + p]
        xt8_c = np.ascontiguousarray(
            x8_pad.reshape(L, 2, P).transpose(2, 0, 1)
        )
        xt8_all.append(xt8_c)
        xn_all.append(xn_c)
        bc_all.append(bc_c)
    print(f"[kernel] host gather: {_time.time()-_tg:.1f}s", flush=True)

    W1s = (W1 * W1_SCALE).astype(F8)  # [256, 256], scaled into e4m3 range
    w18 = W1s.reshape(2, P, H).transpose(1, 0, 2)  # [p, i, m_full]
    consts = {
        "w18a": np.ascontiguousarray(w18[:, :, 0:P]),
        "w18b": np.ascontiguousarray(w18[:, :, P:H]),
        "w2a": W2[0:P, :].astype(BF16),
        "w2b": W2[P:H, :].astype(BF16),
        "b1a": b1[0:P, None].astype(F32),
        "b1b": b1[P:H, None].astype(F32),
        "b2c": np.full((P, 1), b2[0] if b2.ndim else b2, dtype=F32),
        "iota": np.tile(np.arange(P, dtype=F32), (P, 1)).astype(BF16),
    }

    in_maps = [
        {"xt8": xt8_all[c], "xn": xn_all[c], "bc": bc_all[c], **consts}
        for c in range(N_CORES)
    ]
    return T_blk, in_maps


def kernel(x, batch, num_graphs, W1, b1, W2, b2):
    import time as _time

    ng = int(num_graphs)
    assert ng == G
    T_blk, in_maps = prepare_inputs(x, batch, W1, b1, W2, b2)

    t0 = _time.time()
    nc = _build_program(T_blk)
    _split_sync_waits(nc)
    print(f"[kernel] build+split: {_time.time()-t0:.1f}s (T_blk={T_blk})", flush=True)

    t0 = _time.time()
    _run_warmup()
    print(f"[kernel] warmup run: {_time.time()-t0:.1f}s", flush=True)

    t0 = _time.time()
    trace = os.environ.get("KERNEL_TRACE", "0") == "1"
    if trace:
        _install_ntff_hook_shim()
    res = _run_spmd_retry(
        nc, in_maps, list(range(N_CORES)), "main", trace=trace
    )
    print(f"[kernel] main run (compile+upload+exec): {_time.time()-t0:.1f}s", flush=True)
    if trace:
        global LAST_EXEC_NS, LAST_TRACE_PATH
        LAST_EXEC_NS = res.exec_time_ns
        if res.instructions_and_trace is not None:
            LAST_TRACE_PATH = res.instructions_and_trace[1]
        print(f"[kernel] exec_time_ns={res.exec_time_ns} trace={LAST_TRACE_PATH}",
              flush=True)

    out = np.concatenate([res.results[c]["out"] for c in range(N_CORES)], axis=0)
    return out.astype(F32)


# revision 45
# speedup vs baseline: 4.0139x; 1.0020x over previous
"""AttentionPooling kernel for 8 Trainium2 NeuronCores.

Computation (per graph g): out[g] = sum_i softmax(logits)_i * x_i over nodes i in g,
where logits = tanh(x @ W1 + b1) @ W2 + b2.

Strategy:
- logits are bounded (|logit| <= sum|W2| + |b2| < 17), so exp() is safe without the
  max-subtraction: w_i = e_i / sum(e) with e_i = exp(logit_i). Single pass over x.
- Shard 8192 graphs across 8 cores (1024 each). Per core, 8 "graph blocks" of 128
  graphs; a block's 128 graphs map to the 128 PSUM partitions of an accumulator.
- batch is known on host: node ranges per block are computed on host and the x rows
  are gathered per (core, block) into fixed-size slabs of T_blk*128 rows, so all 8
  cores run one identical program (SPMD).
- x is shipped twice, fp8 both times (68MB total vs 131MB for the bf16 baseline),
  in DMA-friendly layouts (>=2KB contiguous lines per partition, two groups per
  DMA instruction): transposed+packed fp8e4m3 [128, L, 2] for the MLP matmul
  (DoubleRow perf mode contracts K=256 in one matmul), and fp8e3m4
  [128, T_tot, 257] node-major (with ones column) for the pooling matmul
  (PE takes mixed bf16 lhsT x fp8 rhs). Measured end-to-end rel err 1.69e-2
  (gate 2e-2), bit-matching the numpy simulation of the same quantization chain.
- W1 is scaled by 16 into fp8e4m3 normal range; the 1/16 is folded into the tanh
  activation's scale operand.
- Per group of 8 subtiles (1024 nodes): 4 DoubleRow matmuls -> hT in PSUM
  [128, 1024] x2 halves; 2 wide tanh ACTs -> th bf16; 16 tiny lg matmuls
  (th chunk stationary, W2 half moving) -> logit column [128 nodes, 1] each;
  exp ACT [128, 8] -> e; per subtile: onehot_e = (iota==bc)*e (DVE) and
  numer[g, 0:257] += onehot_e.T @ [x | 1] (PE, PSUM accum; col 256 is the
  softmax denominator). Block epilogue divides and DMAs out.
- The group loop is software-pipelined 8 steps deep (DMA / W1-A / W1-B /
  tanh-A / tanh-B / lg / exp / oh / numer) with the two h-halves staggered a
  step apart, so every engine's in-order queue has ready work each step and
  single-buffered PSUM h tiles never stall the PE behind the ACT.
"""

import math
import os
from contextlib import ExitStack

import numpy as np
import ml_dtypes

try:
    import concourse.bass as bass
except ImportError:  # fallback if PYTHONPATH lacks the repo
    import sys

    sys.path.insert(0, "/opt/trn_rl_repo")
    import concourse.bass as bass

import concourse.tile as tile
from concourse import bass_utils, mybir

BF16 = ml_dtypes.bfloat16
F8 = ml_dtypes.float8_e4m3
F8E3 = ml_dtypes.float8_e3m4
F32 = np.float32

LAST_EXEC_NS = None
LAST_TRACE_PATH = None

N_CORES = 8
N_NODES = 1_000_000
H = 256  # hidden
G = 8192  # num graphs
GPC = G // N_CORES  # graphs per core = 1024
GPB = 128  # graphs per block (= PSUM partitions)
BPC = GPC // GPB  # blocks per core = 8
P = 128  # partitions / nodes per subtile
GRP = 8  # subtiles per group (1024 nodes)
W1_SCALE = 16.0


def _split_sync_waits(nc, maxw: int = 1) -> int:
    """The walrus build in this container rejects instructions carrying more
    than one sync-wait. Hoist extra waits onto NoOps inserted just before the
    instruction (same engine, same order => identical semantics)."""
    cnt = 0
    for f in nc.m.functions:
        for bb in f.blocks:
            insts = bb.instructions
            out = []
            changed = False
            for ins in insts:
                si = ins.sync_info
                if si is not None and len(si.on_wait) > maxw:
                    waits = list(si.on_wait)
                    keep, extra = waits[-maxw:], waits[:-maxw]
                    for w in extra:
                        cnt += 1
                        nop = mybir.InstNoOp(
                            name=f"wsplit-{cnt}",
                            engine=ins.engine,
                            sync_info=mybir.SyncInfo(on_wait=[w], on_update=[]),
                            bass_nofuse=True,
                        )
                        nc.register_instruction(nop, overwrite=True)
                        out.append(nop)
                    ins.sync_info = mybir.SyncInfo(
                        on_wait=keep, on_update=si.on_update
                    )
                    changed = True
                out.append(ins)
            if changed:
                bb.instructions = out
    return cnt


def _build_program(T_blk: int):
    nc = bass.Bass("TRN2", target_bir_lowering=False)
    T_tot = BPC * T_blk  # subtiles per core; divisible by GRP since BPC=8
    L = T_tot * P  # node slots per core
    NG = T_tot // GRP  # groups per core
    assert T_tot % GRP == 0

    f32 = mybir.dt.float32
    bf16 = mybir.dt.bfloat16
    f8e4 = mybir.dt.float8e4
    f8e3 = mybir.dt.float8e3

    xt8_d = nc.declare_dram_parameter("xt8", [P, L, 2], f8e4, isOutput=False)
    xn_d = nc.declare_dram_parameter("xn", [P, T_tot, H + 1], f8e3, isOutput=False)
    bc_d = nc.declare_dram_parameter("bc", [P, T_tot], f32, isOutput=False)
    w18a_d = nc.declare_dram_parameter("w18a", [P, 2, P], f8e4, isOutput=False)
    w18b_d = nc.declare_dram_parameter("w18b", [P, 2, P], f8e4, isOutput=False)
    w2a_d = nc.declare_dram_parameter("w2a", [P, 1], bf16, isOutput=False)
    w2b_d = nc.declare_dram_parameter("w2b", [P, 1], bf16, isOutput=False)
    b1a_d = nc.declare_dram_parameter("b1a", [P, 1], f32, isOutput=False)
    b1b_d = nc.declare_dram_parameter("b1b", [P, 1], f32, isOutput=False)
    b2c_d = nc.declare_dram_parameter("b2c", [P, 1], f32, isOutput=False)
    iota_d = nc.declare_dram_parameter("iota", [P, P], bf16, isOutput=False)
    out_d = nc.declare_dram_parameter("out", [GPC, H], f32, isOutput=True)

    Tanh = mybir.ActivationFunctionType.Tanh
    Exp = mybir.ActivationFunctionType.Exp
    EQ = mybir.AluOpType.is_equal
    MUL = mybir.AluOpType.mult
    ADD = mybir.AluOpType.add
    DR = mybir.MatmulPerfMode.DoubleRow

    NGRP = GRP * P  # nodes per group = 1024

    with tile.TileContext(nc) as tc:
        with ExitStack() as ctx:
            consts = ctx.enter_context(tc.tile_pool(name="consts", bufs=1))
            xtpool = ctx.enter_context(tc.tile_pool(name="xt", bufs=4))
            xnpool = ctx.enter_context(tc.tile_pool(name="xn", bufs=9))
            thpool = ctx.enter_context(tc.tile_pool(name="th", bufs=6))
            epool = ctx.enter_context(tc.tile_pool(name="e", bufs=4))
            ohpool = ctx.enter_context(tc.tile_pool(name="oh", bufs=24))
            outpool = ctx.enter_context(tc.tile_pool(name="outp", bufs=2))
            # PSUM budget (8 banks of 2KB): ht 1 buf x 2 tags x [128,1024]f32
            # (2 banks each) = 4, numer 2 x [128,257]f32 = 2, lg 2 x [128,8] = 2.
            ps_ht = ctx.enter_context(
                tc.tile_pool(name="ps_ht", bufs=1, space=bass.MemorySpace.PSUM)
            )
            ps_lg = ctx.enter_context(
                tc.tile_pool(name="ps_lg", bufs=2, space=bass.MemorySpace.PSUM)
            )
            ps_nm = ctx.enter_context(
                tc.tile_pool(name="ps_nm", bufs=2, space=bass.MemorySpace.PSUM)
            )

            # ---- constants (loaded once) ----
            w18a_t = consts.tile([P, 2, P], f8e4)
            nc.sync.dma_start(w18a_t[:], w18a_d[:])
            w18b_t = consts.tile([P, 2, P], f8e4)
            nc.sync.dma_start(w18b_t[:], w18b_d[:])
            w2a_t = consts.tile([P, 1], bf16)
            nc.sync.dma_start(w2a_t[:], w2a_d[:])
            w2b_t = consts.tile([P, 1], bf16)
            nc.sync.dma_start(w2b_t[:], w2b_d[:])
            b1a_t = consts.tile([P, 1], f32)
            nc.sync.dma_start(b1a_t[:], b1a_d[:])
            b1b_t = consts.tile([P, 1], f32)
            nc.sync.dma_start(b1b_t[:], b1b_d[:])
            b2c_t = consts.tile([P, 1], f32)
            nc.sync.dma_start(b2c_t[:], b2c_d[:])
            iota_t = consts.tile([P, P], bf16)
            nc.sync.dma_start(iota_t[:], iota_d[:])
            bc_t = consts.tile([P, T_tot], f32)
            nc.sync.dma_start(bc_t[:], bc_d[:])

            xt_tiles = {}
            xn_tiles = {}
            ht_tiles = {}
            th_tiles = {}
            lg_tiles = {}
            lg_work = {}
            lg_pair = {}
            e_tiles = {}
            oh_tiles = {}
            numer_ref = [None]

            def stage_dma(g):
                # fetch two groups per DMA instruction (4KB contiguous
                # per-partition lines) to halve packet count
                w = 2 if g + 1 < NG else 1
                xt8 = xtpool.tile([P, 2 * NGRP, 2], f8e4, tag="xt8")
                nc.sync.dma_start(
                    xt8[:, 0 : w * NGRP, :],
                    xt8_d[:, g * NGRP : (g + w) * NGRP, :],
                )
                xnt = xnpool.tile([P, 2 * GRP, H + 1], f8e3, tag="xnt")
                nc.sync.dma_start(
                    xnt[:, 0 : w * GRP, :],
                    xn_d[:, g * GRP : (g + w) * GRP, :],
                )
                for k in range(w):
                    xt_tiles[g + k] = xt8[:, k * NGRP : (k + 1) * NGRP, :]
                    xn_tiles[g + k] = xnt[:, k * GRP : (k + 1) * GRP, :]

            def stage_w1A(g):
                xt8 = xt_tiles[g]
                htA = ps_ht.tile([P, NGRP], f32, tag="htA")
                ht_tiles[("A", g)] = htA
                for q in range(2):
                    s, e = q * 512, (q + 1) * 512
                    rhs = xt8[:, s:e, :].rearrange("p n i -> p i n")
                    nc.tensor.matmul(
                        htA[:, s:e], w18a_t[:], rhs,
                        start=True, stop=True, perf_mode=DR,
                        skip_group_check=True,
                    )

            def stage_w1B(g):
                xt8 = xt_tiles.pop(g)
                htB = ps_ht.tile([P, NGRP], f32, tag="htB")
                ht_tiles[("B", g)] = htB
                for q in range(2):
                    s, e = q * 512, (q + 1) * 512
                    rhs = xt8[:, s:e, :].rearrange("p n i -> p i n")
                    nc.tensor.matmul(
                        htB[:, s:e], w18b_t[:], rhs,
                        start=True, stop=True, perf_mode=DR,
                        skip_group_check=True,
                    )

            def stage_tanhA(g):
                htA = ht_tiles.pop(("A", g))
                thA = thpool.tile([P, NGRP], bf16, tag="thA")
                nc.scalar.activation(
                    thA[:], htA[:], Tanh, bias=b1a_t[:], scale=1.0 / W1_SCALE
                )
                th_tiles[("A", g)] = thA

            def stage_tanhB(g):
                htB = ht_tiles.pop(("B", g))
                thB = thpool.tile([P, NGRP], bf16, tag="thB")
                nc.scalar.activation(
                    thB[:], htB[:], Tanh, bias=b1b_t[:], scale=1.0 / W1_SCALE
                )
                th_tiles[("B", g)] = thB

            def emit_lg(g, ii):
                thA, thB, lg = lg_work[g]
                nc.tensor.matmul(
                    lg[:, ii : ii + 1],
                    thA[:, ii * P : (ii + 1) * P],
                    w2a_t[:],
                    start=True, stop=False, skip_group_check=True,
                )
                nc.tensor.matmul(
                    lg[:, ii : ii + 1],
                    thB[:, ii * P : (ii + 1) * P],
                    w2b_t[:],
                    start=False, stop=True, skip_group_check=True,
                )

            def stage_lg_prep(g):
                thA = th_tiles.pop(("A", g))
                thB = th_tiles.pop(("B", g))
                lg = ps_lg.tile([P, GRP], f32, tag="lg")
                lg_tiles[g] = lg
                lg_work[g] = (thA, thB, lg)

            def stage_exp(g):
                lg = lg_tiles.pop(g)
                ecols = epool.tile([P, GRP], f32, tag="ecols")
                nc.scalar.activation(ecols[:], lg[:], Exp, bias=b2c_t[:])
                e_tiles[g] = ecols

            def stage_oh(g):
                ecols = e_tiles.pop(g)
                ohs = []
                for gi in range(GRP):
                    j = g * GRP + gi
                    oh = ohpool.tile([P, P], bf16, tag="oh")
                    nc.vector.tensor_scalar(
                        oh[:], iota_t[:], bc_t[:, j : j + 1],
                        ecols[:, gi : gi + 1], EQ, MUL,
                    )
                    ohs.append(oh)
                return ohs

            def stage_numer(g, ohs, g_lg):
                """Numer matmuls for group g, with group g_lg's tiny lg
                matmuls interleaved between them: an accumulating matmul's
                next LDWEIGHTS otherwise waits for the array to drain
                (~2x the stream time); the short lg matmuls fill those
                drain gaps with useful PE work."""
                xnt = xn_tiles.pop(g)
                for gi in range(GRP):
                    j = g * GRP + gi
                    blk, t_in = divmod(j, T_blk)
                    if t_in == 0:
                        numer = ps_nm.tile([P, H + 1], f32, tag="numer")
                        numer_ref[0] = numer
                    numer = numer_ref[0]
                    nc.tensor.matmul(
                        numer[:],
                        ohs[gi][:],
                        xnt[:, gi, :],
                        start=(t_in == 0),
                        stop=(t_in == T_blk - 1),
                        skip_group_check=True,
                    )
                    if g_lg is not None:
                        emit_lg(g_lg, gi)
                    if t_in == T_blk - 1:
                        # block epilogue: out[g] = numer[g, :256] / numer[g, 256]
                        dn = epool.tile([P, 1], f32, tag="dn")
                        nc.vector.tensor_scalar(
                            dn[:], numer[:, H : H + 1], 1e-30, None, ADD
                        )
                        rec = epool.tile([P, 1], f32, tag="rec")
                        nc.vector.reciprocal(rec[:], dn[:])
                        outt = outpool.tile([P, H], f32, tag="outt")
                        nc.vector.tensor_scalar(
                            outt[:], numer[:, 0:H], rec[:], None, MUL
                        )
                        nc.sync.dma_start(
                            out_d[blk * GPB : (blk + 1) * GPB, :], outt[:]
                        )

            # ---- software-pipelined group loop ----
            # Half-staggered: the B half of each group runs one step behind
            # the A half so that no step's W1 matmuls wait on a tanh issued
            # in the same step (single-buffered PSUM h tiles). The oh stage
            # has its own step so the DVE starts each step with zero
            # unsatisfied dependencies (its ecols were produced last step).
            for s in range(NG + 8):
                gN = s - 8  # numer matmuls (+ block epilogue)
                gO = s - 7  # onehot*e on DVE
                gE = s - 5  # exp
                gL = s - 4  # lg matmuls
                gTB = s - 3  # tanh half B
                gTA = s - 2  # tanh half A
                gWB = s - 2  # W1 half B
                gWA = s - 1  # W1 half A
                gD = s  # DMA in

                if gO >= 0 and gO < NG:
                    oh_tiles[gO] = stage_oh(gO)
                if gE >= 0 and gE < NG:
                    stage_exp(gE)
                has_lg = gL >= 0 and gL < NG
                if has_lg:
                    stage_lg_prep(gL)
                if gN >= 0:
                    stage_numer(gN, oh_tiles.pop(gN), gL if has_lg else None)
                elif has_lg:
                    for ii in range(GRP):
                        emit_lg(gL, ii)
                if has_lg:
                    lg_work.pop(gL)
                if gWA >= 0 and gWA < NG:
                    stage_w1A(gWA)
                if gWB >= 0 and gWB < NG:
                    stage_w1B(gWB)
                if gTA >= 0 and gTA < NG:
                    stage_tanhA(gTA)
                if gTB >= 0 and gTB < NG:
                    stage_tanhB(gTB)
                if gD < NG and gD % 2 == 0:
                    stage_dma(gD)

    return nc


def _install_ntff_hook_shim():
    """This image's antenv lacks axon_hooks, so bass_utils' trace=True path
    can't find the NTFF profile hook trn_boot would register. Provide the
    module and register the ctypes hook ourselves (trace runs only)."""
    import sys as _sys
    import types as _types

    if "antenv.axon_hooks" in _sys.modules:
        return
    import antenv

    mod = _types.ModuleType("antenv.axon_hooks")
    mod._hook = None

    def set_axon_ntff_profile_hook(h):
        mod._hook = h

    def get_axon_ntff_profile_hook():
        return mod._hook

    mod.set_axon_ntff_profile_hook = set_axon_ntff_profile_hook
    mod.get_axon_ntff_profile_hook = get_axon_ntff_profile_hook
    _sys.modules["antenv.axon_hooks"] = mod
    antenv.axon_hooks = mod

    from trn_agent_boot.trn_boot import _ntff_profile_via_ctypes

    hook = _ntff_profile_via_ctypes("/opt/axon/libaxon_pjrt.so")
    if hook is not None:
        set_axon_ntff_profile_hook(hook)


def _run_spmd_retry(nc, in_maps, core_ids, label, tries=6, delay=25.0, **kw):
    """The tunneled device intermittently reports NRT_EXEC_UNIT_UNRECOVERABLE
    right after a previous process's close; it self-recovers within ~1-2 min.
    Retry with backoff instead of dying."""
    import time as _time

    for attempt in range(tries):
        try:
            return bass_utils.run_bass_kernel_spmd(nc, in_maps, core_ids, **kw)
        except Exception as e:  # noqa: BLE001
            msg = str(e)
            transient = (
                "UNRECOVERABLE" in msg
                or "UNAVAILABLE" in msg
                or "NRT_TIMEOUT" in msg
                or "PassThrough failed" in msg
            )
            if not transient or attempt == tries - 1:
                raise
            print(
                f"[kernel] {label}: transient device error "
                f"(attempt {attempt+1}/{tries}), retrying in {delay:.0f}s",
                flush=True,
            )
            _time.sleep(delay)
    raise RuntimeError("unreachable")


def _run_warmup():
    """Run a tiny NEFF touching every engine/op first. The first NEFF executed
    in a fresh process has been observed to hang when it contains the full
    pipeline (ACT table staging race?); a small warmup run avoids it."""
    f32 = mybir.dt.float32
    Tanh = mybir.ActivationFunctionType.Tanh
    Exp = mybir.ActivationFunctionType.Exp
    EQ = mybir.AluOpType.is_equal
    MUL = mybir.AluOpType.mult
    nc = bass.Bass("TRN2", target_bir_lowering=False)
    x_d = nc.declare_dram_parameter("x", [P, P], f32, isOutput=False)
    y_d = nc.declare_dram_parameter("y", [P, P], f32, isOutput=True)
    with tile.TileContext(nc) as tc:
        with ExitStack() as ctx:
            pool = ctx.enter_context(tc.tile_pool(name="p", bufs=2))
            ps = ctx.enter_context(
                tc.tile_pool(name="ps", bufs=1, space=bass.MemorySpace.PSUM)
            )
            t = pool.tile([P, P], f32)
            nc.sync.dma_start(t[:], x_d[:])
            acc = ps.tile([P, P], f32)
            nc.tensor.matmul(acc[:], t[:], t[:], start=True, stop=True)
            t2 = pool.tile([P, P], f32)
            nc.scalar.activation(t2[:], acc[:], Tanh, bias=t[:, 0:1])
            t3 = pool.tile([P, P], f32)
            nc.scalar.activation(t3[:], t2[:], Exp, bias=t[:, 0:1])
            t4 = pool.tile([P, P], f32)
            nc.vector.tensor_scalar(t4[:], t3[:], t[:, 0:1], t[:, 1:2], EQ, MUL)
            t5 = pool.tile([P, 1], f32)
            nc.vector.reciprocal(t5[:], t3[:, 0:1])
            nc.vector.tensor_scalar(t4[:, 0:1], t5[:], t5[:], None, MUL)
            nc.sync.dma_start(y_d[:], t4[:])
    _split_sync_waits(nc)
    xw = np.zeros((P, P), np.float32)
    _run_spmd_retry(
        nc, [{"x": xw} for _ in range(N_CORES)], list(range(N_CORES)), "warmup"
    )


def prepare_inputs(x, batch, W1, b1, W2, b2):
    """Host-side segmentation + per-core gather. Returns (T_blk, in_maps)."""
    x = np.asarray(x, dtype=F32)
    batch = np.asarray(batch).astype(np.int64)
    W1 = np.asarray(W1, dtype=F32)
    b1 = np.asarray(b1, dtype=F32)
    W2 = np.asarray(W2, dtype=F32)
    b2 = np.asarray(b2, dtype=F32)
    assert x.shape == (N_NODES, H) and batch.shape == (N_NODES,)

    # ---- host-side segmentation ----
    block_starts = np.searchsorted(batch, np.arange(0, G + 1, GPB)).astype(np.int64)
    cnts = np.diff(block_starts)
    T_blk = max(1, int(math.ceil(cnts.max() / P)))
    T_tot = BPC * T_blk
    L = T_tot * P

    import time as _time

    _tg = _time.time()
    # full-array dtype conversions once (fast contiguous casts)
    x83 = x.astype(F8E3)
    x8 = x.astype(F8)

    xt8_all = []
    xn_all = []
    bc_all = []
    for c in range(N_CORES):
        x83_pad = np.zeros((L, H), dtype=F8E3)
        x8_pad = np.zeros((L, H), dtype=F8)
        bc_c = np.full((P, T_tot), -1.0, dtype=F32)
        for b in range(BPC):
            gblk = c * BPC + b
            s = int(block_starts[gblk])
            e = min(s + T_blk * P, N_NODES)
            n = e - s
            if n <= 0:
                continue
            r0 = b * T_blk * P
            x83_pad[r0 : r0 + n] = x83[s:e]
            x8_pad[r0 : r0 + n] = x8[s:e]
            vals = np.full(T_blk * P, -1.0, dtype=F32)
            vals[:n] = (batch[s:e] - gblk * GPB).astype(F32)
            bc_c[:, b * T_blk : (b + 1) * T_blk] = vals.reshape(T_blk, P).T
        # xn layout [128, T_tot, 257]: row (p, t) = [x[node t*128+p], 1.0]
        xn_c = np.ones((P, T_tot, H + 1), dtype=F8E3)
        xn_c[:, :, 0:H] = x83_pad.reshape(T_tot, P, H).transpose(1, 0, 2)
        # xt8 layout [128, L, 2]: xt8[p, n, i] = x8[n, 128i + p]
        xt8_c = np.ascontiguousarray(
            x8_pad.reshape(L, 2, P).transpose(2, 0, 1)
        )
        xt8_all.append(xt8_c)
        xn_all.append(xn_c)
        bc_all.append(bc_c)
    print(f"[kernel] host gather: {_time.time()-_tg:.1f}s", flush=True)

    W1s = (W1 * W1_SCALE).astype(F8)  # [256, 256], scaled into e4m3 range
    w18 = W1s.reshape(2, P, H).transpose(1, 0, 2)  # [p, i, m_full]
    consts = {
        "w18a": np.ascontiguousarray(w18[:, :, 0:P]),
        "w18b": np.ascontiguousarray(w18[:, :, P:H]),
        "w2a": W2[0:P, :].astype(BF16),
        "w2b": W2[P:H, :].astype(BF16),
        "b1a": b1[0:P, None].astype(F32),
        "b1b": b1[P:H, None].astype(F32),
        "b2c": np.full((P, 1), b2[0] if b2.ndim else b2, dtype=F32),
        "iota": np.tile(np.arange(P, dtype=F32), (P, 1)).astype(BF16),
    }

    in_maps = [
        {"xt8": xt8_all[c], "xn": xn_all[c], "bc": bc_all[c], **consts}
        for c in range(N_CORES)
    ]
    return T_blk, in_maps


def kernel(x, batch, num_graphs, W1, b1, W2, b2):
    import time as _time

    ng = int(num_graphs)
    assert ng == G
    T_blk, in_maps = prepare_inputs(x, batch, W1, b1, W2, b2)

    t0 = _time.time()
    nc = _build_program(T_blk)
    _split_sync_waits(nc)
    print(f"[kernel] build+split: {_time.time()-t0:.1f}s (T_blk={T_blk})", flush=True)

    t0 = _time.time()
    _run_warmup()
    print(f"[kernel] warmup run: {_time.time()-t0:.1f}s", flush=True)

    t0 = _time.time()
    trace = os.environ.get("KERNEL_TRACE", "0") == "1"
    if trace:
        _install_ntff_hook_shim()
    res = _run_spmd_retry(
        nc, in_maps, list(range(N_CORES)), "main", trace=trace
    )
    print(f"[kernel] main run (compile+upload+exec): {_time.time()-t0:.1f}s", flush=True)
    if trace:
        global LAST_EXEC_NS, LAST_TRACE_PATH
        LAST_EXEC_NS = res.exec_time_ns
        if res.instructions_and_trace is not None:
            LAST_TRACE_PATH = res.instructions_and_trace[1]
        print(f"[kernel] exec_time_ns={res.exec_time_ns} trace={LAST_TRACE_PATH}",
              flush=True)

    out = np.concatenate([res.results[c]["out"] for c in range(N_CORES)], axis=0)
    return out.astype(F32)
